# revision 2
# baseline (speedup 1.0000x reference)
"""Head-sharded (tensor-parallel) causal multi-head attention for 8 TRN2
NeuronCores, written in Bass/Tile.

Contract: kernel(**inputs) takes the FULL inputs of the reference
nn.MultiHeadAttention problem (x [2,2048,2048], mask [1,1,2048,2048] causal,
Wq/Wk/Wv/Wo [2048,2048], bq/bk/bv/bo [2048]) and returns the FULL
(output [2,2048,2048], attn_weights [2,16,2048,2048]) pair.

Sharding: 16 heads split 2-per-core (W_q/W_k/W_v column-sharded, W_o
row-sharded); partial outputs are summed with an on-chip ReduceScatter and
concatenated on the host. attn_weights shards concatenate along the head dim.

All matmuls run in float32r (TF32-like, ~2e-4 relative error) on the
TensorEngine; softmax runs fp32 on ACT/DVE. Causality is exploited: only
lower-triangle 128x128 score blocks are computed; the strict upper triangle
of attn_weights is left as (pre-zeroed) DRAM, which matches softmax(-inf)=0
exactly. bv/bo enter the output as the constant row bv@Wo + bo, added on the
host (exact: attention rows sum to 1, so A@(1 x bv) @ Wo = 1 x (bv @ Wo)).
"""

import json as _json
from contextlib import ExitStack

import numpy as np

# ---------------------------------------------------------------------------
# Problem constants (hardcoded per the harness contract)
# ---------------------------------------------------------------------------
B, S, D_MODEL, NUM_HEADS, N_CORES = 2, 2048, 2048, 16, 8
DK = 128
HPC = NUM_HEADS // N_CORES      # heads per core
DKC = HPC * DK                  # projected dims per core
TOK = B * S

# ---------------------------------------------------------------------------
# Workarounds for this walrus build: every instruction may carry at most ONE
# semaphore wait ("Too many sync wait commands"). Split extra waits onto
# single-wait EventSemaphore carriers inserted just before, on the same
# engine queue. The Tile kernel-tail drain is rebuilt the same way.
# ---------------------------------------------------------------------------
_PATCHED = False


def _apply_patches():
    global _PATCHED
    if _PATCHED:
        return
    import bass_rust
    import concourse.bass as cbass
    import concourse.tile as ctile

    def _patched_drain_and_barrier(self, tick_clock, wait_clock):
        from concourse.vector_clock import ScopedClock

        nc = self.nc
        drain_inst = nc.sync.drain()
        wait_clock.add_sem_waits(
            drain_inst.ins, ScopedClock({None: tick_clock.global_clock})
        )
        si = drain_inst.ins.sync_info
        if si is not None and len(si.on_wait) > 1:
            waits = list(si.on_wait)
            si.on_wait = waits[:1]
            rest = waits[1:]
            while rest:
                chunk, rest = rest[:1], rest[1:]
                extra = nc.sync.drain()
                esi = extra.ins.sync_info
                if esi is None:
                    extra.ins.sync_info = bass_rust.SyncInfo(
                        on_update=[], on_wait=chunk
                    )
                else:
                    esi.on_wait = chunk
        nc.all_engine_barrier()
        assert self.sems is not None
        popped = nc._tile_sem_poison_stack.pop()
        assert popped is self._sem_poison
        nc.clear_and_free_semaphores(list(self.sems.allocated().values()))
        nc.all_engine_barrier()

    def _split_waits(m):
        n = 0
        for f in m.get("functions", []):
            for blk in f.get("blocks", []):
                insts = blk.get("instructions")
                if not insts:
                    continue
                out = []
                for inst in insts:
                    si = inst.get("sync_info")
                    ow = (si or {}).get("on_wait") or []
                    if len(ow) > 1:
                        extra, keep = ow[:-1], ow[-1:]
                        for i, w in enumerate(extra):
                            out.append(
                                {
                                    "debug": inst.get("debug", 0),
                                    "engine": inst["engine"],
                                    "ins": [],
                                    "name": f"WS{n}-{i}-{inst['name']}",
                                    "opcode": "EventSemaphore",
                                    "outs": [],
                                    "sync_info": {
                                        "on_update": [],
                                        "on_wait": [w],
                                    },
                                }
                            )
                        si["on_wait"] = keep
                        n += 1
                    out.append(inst)
                blk["instructions"] = out
        return m

    orig_to_json = cbass.Bass.to_json_bytes

    def _patched_to_json(self, *a, **k):
        return _json.dumps(_split_waits(_json.loads(orig_to_json(self, *a, **k)))).encode()

    ctile.TileContext._drain_and_barrier = _patched_drain_and_barrier
    cbass.Bass.to_json_bytes = _patched_to_json
    _PATCHED = True


# ---------------------------------------------------------------------------
# Device kernel builder
# ---------------------------------------------------------------------------
def _build_nc():
    import concourse.bass as bass
    import concourse.mybir as mybir
    import concourse.tile as tile
    from concourse.masks import make_causal_mask, make_identity

    F32 = mybir.dt.float32
    mm_dt = mybir.dt.float32r
    EXPF = mybir.ActivationFunctionType.Exp
    ADD = mybir.AluOpType.add

    D = D_MODEL
    NT = TOK // 128
    NTS = S // 128
    NSUP = TOK // 256
    NBH = B * HPC
    NPAIR = NTS // 2
    ND = D // 128
    scale = 1.0 / (DK ** 0.5)

    nc = bass.Bass(num_devices=N_CORES)

    x_in = nc.declare_dram_parameter("x", [TOK, D], F32, isOutput=False)
    wq_in = nc.declare_dram_parameter("wq", [D, DKC], F32, isOutput=False)
    wk_in = nc.declare_dram_parameter("wk", [D, DKC], F32, isOutput=False)
    wv_in = nc.declare_dram_parameter("wv", [D, DKC], F32, isOutput=False)
    wo_in = nc.declare_dram_parameter("wo", [DKC, D], F32, isOutput=False)
    bq_in = nc.declare_dram_parameter("bq", [DKC, 1], F32, isOutput=False)
    bk_in = nc.declare_dram_parameter("bk", [DKC, 1], F32, isOutput=False)

    attn_out = nc.declare_dram_parameter("attn", [B, HPC, S, S], F32, isOutput=True)
    out_rs = nc.declare_dram_parameter("out_rs", [TOK // N_CORES, D], F32, isOutput=True)

    out_part = nc.dram_tensor("out_part", [TOK, D], F32)
    rs_int = nc.dram_tensor("rs_int", [TOK // N_CORES, D], F32)

    with tile.TileContext(nc) as tc, ExitStack() as ctx:
        persist = ctx.enter_context(tc.tile_pool(name="persist", bufs=1))

        qt_sb = persist.tile([128, HPC, TOK], mm_dt, tag="qt")
        kt_sb = persist.tile([128, HPC, TOK], mm_dt, tag="kt")
        v_sb = persist.tile([128, NT, DKC], mm_dt, tag="v")
        ident = persist.tile([128, 128], F32, tag="ident")
        dmask = persist.tile([128, 128], F32, tag="dmask")
        bq_sb = persist.tile([128, HPC], F32, tag="bq")
        bk_sb = persist.tile([128, HPC], F32, tag="bk")
        zeros128 = persist.tile([128, 128], F32, tag="zeros128")
        nc.vector.memset(zeros128, 0.0)

        make_identity(nc, ident)
        make_causal_mask(nc, dmask, mask_val=-1e38)
        nc.gpsimd.dma_start(
            out=bq_sb, in_=bq_in[:, :].rearrange("(h p) o -> p (h o)", p=128)
        )
        nc.gpsimd.dma_start(
            out=bk_sb, in_=bk_in[:, :].rearrange("(h p) o -> p (h o)", p=128)
        )

        # --- phase 1: X transposes + QKV projections ---
        with (
            tc.tile_pool(name="p1sb", bufs=1) as p1sb,
            tc.tile_pool(name="p1x", bufs=3) as p1x,
            tc.tile_pool(name="p1xt", bufs=1) as p1xt,
            tc.tile_pool(name="p1ps", bufs=3, space="PSUM") as p1ps,
            tc.tile_pool(name="p1psx", bufs=2, space="PSUM") as p1psx,
        ):
            wq_sb = p1sb.tile([128, ND, DKC], mm_dt, tag="wq")
            wk_sb = p1sb.tile([128, ND, DKC], mm_dt, tag="wk")
            wv_sb = p1sb.tile([128, ND, DKC], mm_dt, tag="wv")
            for w_sb, w_in in ((wq_sb, wq_in), (wk_sb, wk_in), (wv_sb, wv_in)):
                nc.gpsimd.dma_start(
                    out=w_sb, in_=w_in[:, :].rearrange("(j p) c -> p j c", p=128)
                )

            for sup in range(NSUP):
                t0, t1 = 2 * sup, 2 * sup + 1
                x0 = p1x.tile([128, D], F32, tag="x")
                x1 = p1x.tile([128, D], F32, tag="x")
                nc.gpsimd.dma_start(out=x0, in_=x_in[t0 * 128 : (t0 + 1) * 128, :])
                nc.gpsimd.dma_start(out=x1, in_=x_in[t1 * 128 : (t1 + 1) * 128, :])
                xt = p1xt.tile([128, ND, 256], mm_dt, tag="xt")
                for j in range(ND):
                    pxt = p1psx.tile([128, 256], F32, tag="pxt")
                    nc.tensor.transpose(
                        pxt[:, 0:128], x0[:, j * 128 : (j + 1) * 128], ident
                    )
                    nc.tensor.transpose(
                        pxt[:, 128:256], x1[:, j * 128 : (j + 1) * 128], ident
                    )
                    nc.scalar.copy(xt[:, j, :], pxt)
                for w_sb, dst, b_sb in (
                    (wq_sb, qt_sb, bq_sb),
                    (wk_sb, kt_sb, bk_sb),
                ):
                    for m in range(HPC):
                        pq = p1ps.tile([128, 256], F32, tag="pqkv")
                        for j in range(ND):
                            nc.tensor.matmul(
                                pq,
                                w_sb[:, j, m * 128 : (m + 1) * 128],
                                xt[:, j, :],
                                start=(j == 0),
                                stop=(j == ND - 1),
                            )
                        nc.vector.tensor_scalar_add(
                            dst[:, m, sup * 256 : (sup + 1) * 256],
                            pq,
                            b_sb[:, m : m + 1],
                        )
                for ti, xtc in ((t0, (0, 128)), (t1, (128, 256))):
                    pv = p1ps.tile([128, DKC], F32, tag="pqkv")
                    for j in range(ND):
                        nc.tensor.matmul(
                            pv,
                            xt[:, j, xtc[0] : xtc[1]],
                            wv_sb[:, j, :],
                            start=(j == 0),
                            stop=(j == ND - 1),
                        )
                    nc.vector.tensor_copy(out=v_sb[:, ti, :], in_=pv)

        persist2 = ctx.enter_context(tc.tile_pool(name="persist2", bufs=1))
        ctxt_sb = persist2.tile([128, NBH, S], mm_dt, tag="ctxt")
        wo_sb = persist2.tile([128, HPC, D], mm_dt, tag="wo")
        nc.gpsimd.dma_start(
            out=wo_sb, in_=wo_in[:, :].rearrange("(h p) e -> p h e", p=128)
        )

        # --- phase 2: causal attention per (b, head-local) ---
        with (
            tc.tile_pool(name="p2a", bufs=4) as p2a,
            tc.tile_pool(name="p2at", bufs=3) as p2at,
            tc.tile_pool(name="p2sm", bufs=8) as p2sm,
            tc.tile_pool(name="p2ps", bufs=3, space="PSUM") as p2ps,
            tc.tile_pool(name="p2pat", bufs=2, space="PSUM") as p2pat,
            tc.tile_pool(name="p2pcx", bufs=2, space="PSUM") as p2pcx,
        ):
            for b in range(B):
                for hl in range(HPC):
                    bh = b * HPC + hl
                    for p in range(NPAIR):
                        a_tiles = []
                        for qt in (2 * p, 2 * p + 1):
                            nkeys = (qt + 1) * 128
                            nch = (nkeys + 511) // 512
                            a_sb = p2a.tile([128, S], F32, tag="a")
                            sums = p2sm.tile([128, 4], F32, tag="sums")
                            for c in range(nch):
                                c0 = c * 512
                                cw = min(512, nkeys - c0)
                                ps = p2ps.tile([128, 512], F32, tag="ps")
                                nc.tensor.matmul(
                                    ps[:, 0:cw],
                                    qt_sb[:, hl, b * S + qt * 128 : b * S + (qt + 1) * 128],
                                    kt_sb[:, hl, b * S + c0 : b * S + c0 + cw],
                                    start=True,
                                    stop=True,
                                )
                                if c == nch - 1:
                                    nc.vector.tensor_tensor(
                                        out=ps[:, cw - 128 : cw],
                                        in0=ps[:, cw - 128 : cw],
                                        in1=dmask,
                                        op=ADD,
                                    )
                                nc.scalar.activation(
                                    a_sb[:, c0 : c0 + cw],
                                    ps[:, 0:cw],
                                    EXPF,
                                    bias=0.0,
                                    scale=scale,
                                    accum_out=sums[:, c : c + 1],
                                )
                            rowsum = p2sm.tile([128, 1], F32, tag="rowsum")
                            nc.vector.tensor_reduce(
                                rowsum, sums[:, 0:nch], mybir.AxisListType.X, ADD
                            )
                            rinv = p2sm.tile([128, 1], F32, tag="rinv")
                            nc.vector.reciprocal(rinv, rowsum)
                            nc.vector.tensor_scalar_mul(
                                a_sb[:, 0:nkeys], a_sb[:, 0:nkeys], rinv
                            )
                            nc.gpsimd.dma_start(
                                out=attn_out[
                                    b, hl, qt * 128 : (qt + 1) * 128, 0:nkeys
                                ],
                                in_=a_sb[:, 0:nkeys],
                            )
                            a_tiles.append(a_sb)
                        pcx = p2pcx.tile([128, 256], F32, tag="pcx")
                        nj = 2 * p + 2
                        for j in range(nj):
                            pat = p2pat.tile([128, 256], F32, tag="pat")
                            at_sb = p2at.tile([128, 256], mm_dt, tag="at")
                            if j <= 2 * p:
                                nc.tensor.transpose(
                                    pat[:, 0:128],
                                    a_tiles[0][:, j * 128 : (j + 1) * 128],
                                    ident,
                                )
                                nc.tensor.transpose(
                                    pat[:, 128:256],
                                    a_tiles[1][:, j * 128 : (j + 1) * 128],
                                    ident,
                                )
                                nc.scalar.copy(at_sb, pat)
                            else:
                                nc.tensor.transpose(
                                    pat[:, 128:256],
                                    a_tiles[1][:, j * 128 : (j + 1) * 128],
                                    ident,
                                )
                                nc.vector.tensor_copy(
                                    out=at_sb[:, 0:128], in_=zeros128
                                )
                                nc.scalar.copy(at_sb[:, 128:256], pat[:, 128:256])
                            nc.tensor.matmul(
                                pcx,
                                v_sb[:, b * NTS + j, hl * 128 : (hl + 1) * 128],
                                at_sb,
                                start=(j == 0),
                                stop=(j == nj - 1),
                            )
                        nc.vector.tensor_copy(
                            out=ctxt_sb[:, bh, p * 256 : (p + 1) * 256], in_=pcx
                        )

        # --- phase 3: output projection + reduce-scatter ---
        with (
            tc.tile_pool(name="p3o", bufs=3) as p3o,
            tc.tile_pool(name="p3ps", bufs=3, space="PSUM") as p3ps,
        ):
            for t in range(NT):
                b, qt = t // NTS, t % NTS
                o_sb = p3o.tile([128, D], F32, tag="o")
                for nchk in range(D // 512):
                    po = p3ps.tile([128, 512], F32, tag="po")
                    for hl in range(HPC):
                        nc.tensor.matmul(
                            po,
                            ctxt_sb[:, b * HPC + hl, qt * 128 : (qt + 1) * 128],
                            wo_sb[:, hl, nchk * 512 : (nchk + 1) * 512],
                            start=(hl == 0),
                            stop=(hl == HPC - 1),
                        )
                    nc.scalar.copy(o_sb[:, nchk * 512 : (nchk + 1) * 512], po)
                nc.gpsimd.dma_start(out=out_part[t * 128 : (t + 1) * 128, :], in_=o_sb)

            nc.gpsimd.collective_compute(
                "ReduceScatter",
                ADD,
                replica_groups=[list(range(N_CORES))],
                ins=[out_part[:, :]],
                outs=[rs_int[:, :]],
            )
            nc.gpsimd.dma_start(out=out_rs[:, :], in_=rs_int[:, :])

    return nc


# ---------------------------------------------------------------------------
# Host-side execution: sharded PJRT run with cached jit, device-side zero
# outputs (donated), and zero-copy input staging.
# ---------------------------------------------------------------------------
_EXEC_CACHE = {}


def _get_exec():
    if "exec" in _EXEC_CACHE:
        return _EXEC_CACHE["exec"]

    _apply_patches()
    import jax
    import jax.numpy as jnp
    from jax.experimental.shard_map import shard_map
    from jax.sharding import Mesh, NamedSharding, PartitionSpec

    from concourse import bass2jax, mybir

    bass2jax.install_neuronx_cc_hook()
    nc = _build_nc()

    partition_name = nc.partition_id_tensor.name if nc.partition_id_tensor else None
    in_names, out_names, out_avals = [], [], []
    for alloc in nc.m.functions[0].allocations:
        if not isinstance(alloc, mybir.MemoryLocationSet):
            continue
        name = alloc.memorylocations[0].name
        if alloc.kind == "ExternalInput":
            if name != partition_name:
                in_names.append(name)
        elif alloc.kind == "ExternalOutput":
            shape = tuple(alloc.tensor_shape)
            dtype = mybir.dt.np(alloc.dtype)
            out_names.append(name)
            out_avals.append(jax.core.ShapedArray(shape, dtype))
    n_params = len(in_names)
    n_outs = len(out_names)
    all_in_names = list(in_names) + list(out_names)
    if partition_name is not None:
        all_in_names.append(partition_name)
    donate = tuple(range(n_params, n_params + n_outs))

    def _body(*args):
        operands = list(args)
        if partition_name is not None:
            operands.append(bass2jax.partition_id_tensor())
        outs = bass2jax._bass_exec_p.bind(
            *operands,
            out_avals=tuple(out_avals),
            in_names=tuple(all_in_names),
            out_names=tuple(out_names),
            lowering_input_output_aliases=(),
            sim_require_finite=True,
            sim_require_nnan=True,
            nc=nc,
        )
        return tuple(outs)

    devices = jax.devices()[:N_CORES]
    mesh = Mesh(np.asarray(devices), ("core",))
    spec = PartitionSpec("core")
    sharded = jax.jit(
        shard_map(
            _body,
            mesh=mesh,
            in_specs=(spec,) * (n_params + n_outs),
            out_specs=(spec,) * n_outs,
            check_rep=False,
        ),
        donate_argnums=donate,
        keep_unused=True,
    )
    sharding = NamedSharding(mesh, spec)

    def make_zeros():
        return [
            jax.jit(
                lambda s=a.shape, d=a.dtype: jnp.zeros((N_CORES * s[0],) + s[1:], d),
                out_shardings=sharding,
            )()
            for a in out_avals
        ]

    def stage(per_core):
        """per_core: list of dicts name->np array. Returns staged global arrays."""
        staged = []
        for i, name in enumerate(in_names):
            arrs = [np.asarray(m[name]) for m in per_core]
            d0 = arrs[0].shape[0]
            gshape = (N_CORES * d0,) + arrs[0].shape[1:]
            staged.append(
                jax.make_array_from_callback(
                    gshape, sharding, lambda idx, a=arrs, d0=d0: a[idx[0].start // d0]
                )
            )
        return staged

    ex = {
        "sharded": sharded,
        "stage": stage,
        "make_zeros": make_zeros,
        "in_names": in_names,
        "out_names": out_names,
        "out_avals": out_avals,
    }
    _EXEC_CACHE["exec"] = ex
    return ex


def _run_device(per_core_inputs):
    ex = _get_exec()
    staged = ex["stage"](per_core_inputs)
    zeros = ex["make_zeros"]()
    outs = ex["sharded"](*staged, *zeros)
    return dict(zip(ex["out_names"], outs))


def _shard_inputs(x, Wq, bq, Wk, bk, Wv, bv, Wo, bo, mask=None):
    xf = np.ascontiguousarray(np.asarray(x, dtype=np.float32).reshape(TOK, D_MODEL))
    per_core = []
    for c in range(N_CORES):
        sl = slice(c * DKC, (c + 1) * DKC)
        per_core.append(
            {
                "x": xf,
                "wq": np.ascontiguousarray(np.asarray(Wq, np.float32)[:, sl]),
                "wk": np.ascontiguousarray(np.asarray(Wk, np.float32)[:, sl]),
                "wv": np.ascontiguousarray(np.asarray(Wv, np.float32)[:, sl]),
                "wo": np.ascontiguousarray(np.asarray(Wo, np.float32)[sl, :]),
                "bq": np.ascontiguousarray(
                    np.asarray(bq, np.float32)[sl].reshape(DKC, 1)
                ),
                "bk": np.ascontiguousarray(
                    np.asarray(bk, np.float32)[sl].reshape(DKC, 1)
                ),
            }
        )
    return per_core


def kernel(x, mask, Wq, bq, Wk, bk, Wv, bv, Wo, bo):
    """Full-input, full-output MHA forward. Returns (output, attn_weights)."""
    per_core = _shard_inputs(x, Wq, bq, Wk, bk, Wv, bv, Wo, bo, mask)
    outs = _run_device(per_core)

    attn_g = np.asarray(outs["attn"])  # [8*B, HPC, S, S]
    out_g = np.asarray(outs["out_rs"])  # [TOK, D]

    attn = np.empty((B, NUM_HEADS, S, S), dtype=np.float32)
    for c in range(N_CORES):
        attn[:, c * HPC : (c + 1) * HPC] = attn_g[c * B : (c + 1) * B]

    # bv/bo contribution: out += bv @ Wo + bo (exact; attention rows sum to 1)
    const_row = (
        np.asarray(bv, np.float32) @ np.asarray(Wo, np.float32)
        + np.asarray(bo, np.float32)
    ).astype(np.float32)
    output = (out_g + const_row[None, :]).reshape(B, S, D_MODEL)
    return output, attn
